# revision 11
# baseline (speedup 1.0000x reference)
"""DCGRU cell Trainium2 kernel: batch-parallel SPMD over 8 NeuronCores.

Sharding: data-parallel over batch B=16 -> 2 batches/core; supports and
weights replicated. No collectives.

fp8 version: all S-product matmuls run fp8-e4m3 with DoubleRow perf mode
(2 contraction rows per PE cell -> 2x MACs/cycle, and half the support
DMA bytes vs bf16). Scales keep operands in e4m3 range:
  S stored as S*4096 (values ~U[0,1])
  x1 = S@x stored as 64*x1 (std ~0.8); W rows for x1 feature groups
  pre-divided by 64 on host. Dense (W) matmuls + activations stay bf16/f32.

Device algorithm per core (batches b0,b1):
  x = concat([inputs, states], -1)                  [N, 128] per batch
  Phase 0:  load x natural (bf16), PE-transpose -> x^T; DVE cast -> X8 fp8
  Phase A1: psum = 4096*(S_s @ x)^T; t = psum*2^-6 = 64*x1 (bf16) -> h1 DRAM;
            PE-transpose t -> X1 fp8 natural (value 64*x1)
  Phase A2: psum = 2^18*(S_s @ x1)^T; t = psum*2^-17 - x^T = x2^T -> h1
  Phase D1: ru^T = sigmoid(W_ru^T h^T + b), rs^T = r^T * states^T,
            XC2 fp8 natural (batch-packed 2x64 features)
  Phase B1: psum = 4096*(S_s @ rs)^T packed; t = psum*2^-6 -> h2 + XC3 fp8
  Phase B2: psum = 2^18*(S_s @ xc1)^T; t = psum*2^-17 - rs^T -> h2
  Phase D2: c^T = tanh(W_c^T h'^T + b_c), out^T = c + u*(s - c),
            PE-transpose -> natural, DMA out.
"""

import sys

sys.path.insert(0, "/opt/trn_rl_repo")

from contextlib import ExitStack

import ml_dtypes
import numpy as np

import concourse.bacc as bacc
import concourse.bass as bass
import concourse.mybir as mybir
import concourse.tile as tile
from concourse.bass_utils import run_bass_kernel_spmd

BF16 = mybir.dt.bfloat16
FP8 = mybir.dt.float8e4
F32 = mybir.dt.float32
AF = mybir.ActivationFunctionType
ALU = mybir.AluOpType
DR = mybir.MatmulPerfMode.DoubleRow

N = 8192
DC = 128          # D_IN + D_H
H = 64
B2 = 2            # batches per core
NSB = N // 256    # 32 contraction superblocks (DoubleRow: 256 rows each)
CH = 512          # psum chunk (free dim)
NCH = N // CH     # 16 chunks
# groups of chunks sharing one stationary load; 6 product psum banks max
GROUPS = [(0, 3), (3, 3), (6, 3), (9, 3), (12, 3), (15, 1)]
NSUP = 2

S_SCALE = 4096.0   # S stored *4096 in e4m3
X1_SCALE = 64.0    # hop-1 features stored *64

_CACHE = {}


def _build():
    import os
    PHASES = int(os.environ.get("DCGRU_PHASES", "6"))
    nc = bacc.Bacc("TRN2", target_bir_lowering=False, debug=False)

    xct_d = nc.dram_tensor("xcatT", [B2, DC, N], BF16, kind="ExternalInput")
    xc8_d = nc.dram_tensor("xcat8", [B2, N, DC], FP8, kind="ExternalInput")
    # supports^T, *4096, fp8, DoubleRow layout [s, sb, ki, ko, n]
    sup_d = nc.dram_tensor("supT", [NSUP, NSB, 128, 2, N], FP8,
                           kind="ExternalInput")
    wru_d = nc.dram_tensor("wru", [5 * DC, 2 * H], BF16, kind="ExternalInput")
    wc_d = nc.dram_tensor("wc", [5 * DC, H], BF16, kind="ExternalInput")
    bru_d = nc.dram_tensor("bru", [2 * H, 1], F32, kind="ExternalInput")
    bc_d = nc.dram_tensor("bc", [H, 1], F32, kind="ExternalInput")
    out_d = nc.dram_tensor("out", [B2, N, H], F32, kind="ExternalOutput")

    id_bf = nc.inline_tensor(np.eye(128, dtype=ml_dtypes.bfloat16), "id_bf")
    id_f = nc.inline_tensor(np.eye(128, dtype=np.float32), "id_f")

    xct_ap = xct_d.ap()
    xc8_ap = xc8_d.ap()
    sup_ap = sup_d.ap()
    out_ap = out_d.ap()

    with tile.TileContext(nc) as tc, ExitStack() as ctx:
        cpool = ctx.enter_context(tc.tile_pool(name="const", bufs=1))
        dram = ctx.enter_context(tc.tile_pool(name="dram", bufs=1, space="DRAM"))
        pers = ctx.enter_context(tc.tile_pool(name="pers", bufs=1))
        st = ctx.enter_context(tc.tile_pool(name="st", bufs=6))
        stage = ctx.enter_context(tc.tile_pool(name="stage", bufs=10))
        onat = ctx.enter_context(tc.tile_pool(name="onat", bufs=4))
        pp = ctx.enter_context(tc.tile_pool(name="pp", bufs=6, space="PSUM"))
        pt = ctx.enter_context(tc.tile_pool(name="pt", bufs=2, space="PSUM"))

        # ---- constants ----
        IDB = cpool.tile([128, 128], BF16, tag="idb", name="idb")
        nc.sync.dma_start(IDB[:], id_bf.ap())
        IDF = cpool.tile([128, 128], F32, tag="idf", name="idf")
        nc.sync.dma_start(IDF[:], id_f.ap())
        WRU = cpool.tile([128, 5 * 128], BF16, tag="wru", name="wru")
        nc.sync.dma_start(
            WRU[:].rearrange("p (a o) -> p a o", a=5),
            wru_d.ap().rearrange("(a p) o -> p a o", p=128),
        )
        # WC layout: cols m*64:(m+1)*64 = inputs-half block (rows 0:64);
        # cols 320+m*64 = states-half block, duplicated at rows 0:64 and 64:128
        WC = cpool.tile([128, 10 * 64], BF16, tag="wc", name="wc")
        for m in range(5):
            nc.sync.dma_start(
                WC[0:64, m * 64:(m + 1) * 64], wc_d.ap()[m * 128:m * 128 + 64, :]
            )
            nc.sync.dma_start(
                WC[0:64, 320 + m * 64:320 + (m + 1) * 64],
                wc_d.ap()[m * 128 + 64:(m + 1) * 128, :],
            )
            nc.sync.dma_start(
                WC[64:128, 320 + m * 64:320 + (m + 1) * 64],
                wc_d.ap()[m * 128 + 64:(m + 1) * 128, :],
            )
        BRU = cpool.tile([128, 1], F32, tag="bru", name="bru")
        nc.sync.dma_start(BRU[:], bru_d.ap())
        BC = cpool.tile([64, 1], F32, tag="bc", name="bc")
        nc.sync.dma_start(BC[:], bc_d.ap())

        # ---- DRAM scratch: gconv1 product feats^T (x1_s0, x2_s0, x1_s1, x2_s1) ----
        h1 = [[dram.tile([128, N], BF16, tag=f"h1_{b}_{m}", name=f"h1_{b}_{m}") for m in range(4)]
              for b in range(B2)]
        # gconv2 states-half feats^T, batch-packed rows (b*64): [x1'_s, x2'_s]
        h2 = [[dram.tile([128, N], BF16, tag=f"h2_{s}_{k}", name=f"h2_{s}_{k}") for k in range(2)]
              for s in range(NSUP)]

        # ---- persistent SBUF; tags share slots across phases by lifetime ----
        # big4 (fp8): X1 (A1-A2) -> XC2/XC3 (D1-B2)
        XT = [pers.tile([128, N], BF16, tag="xt", name=f"XT_{b}", bufs=2)
              for b in range(B2)]
        X8 = [pers.tile([128, N], FP8, tag="x8", name=f"X8_{b}", bufs=2)
              for b in range(B2)]
        X1 = [[pers.tile([128, N], FP8, tag="big4", name=f"X1_{s}_{b}", bufs=4)
               for b in range(B2)] for s in range(NSUP)]

        def dr_lhs(tile_, sb):
            """[128, 2, 128] DoubleRow stationary view of natural fp8 tile."""
            return tile_[:, sb * 256:(sb + 1) * 256].rearrange(
                "p (i d) -> p i d", i=2)

        # ---- phase 0: x^T (bf16) and x natural (fp8) come straight from host ----
        for b in range(B2):
            nc.sync.dma_start(XT[b][:], xct_ap[b])
            nc.sync.dma_start(
                X8[b][:].rearrange("p (a d) -> p a d", a=N // 128),
                xc8_ap[b].rearrange("(a p) d -> p a d", p=128),
            )

        def product_stream(lhs_of, psum_sink, pack_batches):
            """Stream supT once; for each (s, group, superblock) do DR matmuls.

            lhs_of(s, b, sb) -> lhsT AP [128, 2, 128] fp8. psum_sink(s,
            b_or_None, j, c0, cnt, psum) consumes the finished [128, CH]
            f32 psum for chunk c0+j.
            """
            for s in range(NSUP):
                for (c0, cnt) in GROUPS:
                    gc = cnt * CH
                    if pack_batches:
                        psums = [pp.tile([128, CH], F32, tag="pp", name="pp") for j in range(cnt)]
                    else:
                        psums = [pp.tile([128, CH], F32, tag="pp", name="pp")
                                 for _ in range(B2 * cnt)]
                    for sb in range(NSB):
                        stt = st.tile([128, 2, gc], FP8, tag="st", name="st")
                        nc.sync.dma_start(
                            stt[:],
                            sup_ap[s, sb, :, :, c0 * CH:c0 * CH + gc],
                        )
                        first = sb == 0
                        last = sb == NSB - 1
                        if pack_batches:
                            lhsT = lhs_of(s, None, sb)
                            for j in range(cnt):
                                nc.tensor.matmul(
                                    psums[j][:], lhsT,
                                    stt[:, :, j * CH:(j + 1) * CH],
                                    start=first, stop=last, perf_mode=DR,
                                )
                        else:
                            for b in range(B2):
                                lhsT = lhs_of(s, b, sb)
                                for j in range(cnt):
                                    nc.tensor.matmul(
                                        psums[b * cnt + j][:], lhsT,
                                        stt[:, :, j * CH:(j + 1) * CH],
                                        start=first, stop=last, perf_mode=DR,
                                    )
                    if pack_batches:
                        for j in range(cnt):
                            psum_sink(s, None, j, c0, cnt, psums[j])
                    else:
                        for b in range(B2):
                            for j in range(cnt):
                                psum_sink(s, b, j, c0, cnt, psums[b * cnt + j])

        # ---- A1: psum = 4096*(S_s @ x)^T; t = 64*x1 ----
        def a1_sink(s, b, j, c0, cnt, psum):
            cc = c0 + j
            cols = slice(cc * CH, (cc + 1) * CH)
            t = stage.tile([128, CH], BF16, tag="sg", name="sg")
            nc.scalar.activation(t[:], psum[:], AF.Copy, scale=1.0 / X1_SCALE)
            nc.sync.dma_start(h1[b][2 * s][:, cols], t[:])
            for tp in range(4):
                blk = cc * 4 + tp
                ps = pt.tile([128, 128], BF16, tag="tp", name="tp")
                nc.tensor.transpose(ps[:], t[:, tp * 128:(tp + 1) * 128], IDB[:])
                nc.vector.tensor_copy(
                    X1[s][b][:, blk * 128:(blk + 1) * 128], ps[:]
                )

        product_stream(lambda s, b, sb: dr_lhs(X8[b], sb),
                       a1_sink, pack_batches=False)

        if PHASES < 2:
            return nc
        # ---- A2: psum = 2^18*(S_s @ x1)^T; t = psum*2^-17 - x^T ----
        def a2_sink(s, b, j, c0, cnt, psum):
            cc = c0 + j
            cols = slice(cc * CH, (cc + 1) * CH)
            t = stage.tile([128, CH], BF16, tag="sg", name="sg")
            nc.vector.scalar_tensor_tensor(
                t[:], psum[:], 2.0 / (S_SCALE * X1_SCALE), XT[b][:, cols],
                op0=ALU.mult, op1=ALU.subtract,
            )
            nc.sync.dma_start(h1[b][2 * s + 1][:, cols], t[:])

        product_stream(lambda s, b, sb: dr_lhs(X1[s][b], sb),
                       a2_sink, pack_batches=False)

        if PHASES < 3:
            return nc
        # ---- D1: dense ru + sigmoid + rs^T + XC2 fp8 natural ----
        RUT = [pers.tile([128, N], BF16, tag="big2", name=f"RUT_{b}", bufs=2)
               for b in range(B2)]
        RST = pers.tile([128, N], BF16, tag="rst", name="RST", bufs=1)
        XC2 = pers.tile([128, N], FP8, tag="big4", name="XC2", bufs=4)
        for b in range(B2):
            for cc in range(NCH):
                cols = slice(cc * CH, (cc + 1) * CH)
                ps = pt.tile([128, CH], F32, tag="tp", name="tp")
                for i in range(5):
                    if i == 0:
                        rhs = XT[b][:, cols]
                    else:
                        sg = stage.tile([128, CH], BF16, tag="sg", name="sg")
                        nc.sync.dma_start(sg[:], h1[b][i - 1][:, cols])
                        rhs = sg[:]
                    nc.tensor.matmul(
                        ps[:], WRU[:, i * 128:(i + 1) * 128], rhs,
                        start=(i == 0), stop=(i == 4),
                    )
                nc.scalar.activation(
                    RUT[b][:, cols], ps[:], AF.Sigmoid, bias=BRU[:]
                )
                # rs = r * states^T; base-shift states^T and the result via
                # single-input copies (SB-SB two-input ops need equal bases)
                sts = stage.tile([64, CH], BF16, tag="sh1", name="sh1", bufs=3)
                nc.vector.tensor_copy(sts[:], XT[b][64:128, cols])
                rsc = stage.tile([64, CH], BF16, tag="sh2", name="sh2", bufs=3)
                nc.vector.tensor_mul(rsc[:], RUT[b][0:64, cols], sts[:])
                nc.vector.tensor_copy(RST[b * 64:(b + 1) * 64, cols], rsc[:])
        # XC2 natural: one packed transpose per 128-node block (both batches)
        for cc in range(NCH):
            for tp in range(4):
                blk = cc * 4 + tp
                ps2 = pt.tile([128, 128], BF16, tag="tp", name="tp")
                nc.tensor.transpose(
                    ps2[:], RST[:, blk * 128:(blk + 1) * 128], IDB[:]
                )
                nc.vector.tensor_copy(
                    XC2[:, blk * 128:(blk + 1) * 128], ps2[:]
                )

        if PHASES < 4:
            return nc
        # ---- B1: psum = 4096*(S_s @ rs)^T packed; t = 64*xc1 ----
        XC3 = pers.tile([128, N], FP8, tag="big4", name="XC3", bufs=4)

        def b1_sink(s, b, j, c0, cnt, psum):
            cc = c0 + j
            cols = slice(cc * CH, (cc + 1) * CH)
            t = stage.tile([128, CH], BF16, tag="sg", name="sg")
            nc.scalar.activation(t[:], psum[:], AF.Copy, scale=1.0 / X1_SCALE)
            nc.sync.dma_start(h2[s][0][:, cols], t[:])
            for tp in range(4):
                blk = cc * 4 + tp
                ps = pt.tile([128, 128], BF16, tag="tp", name="tp")
                nc.tensor.transpose(ps[:], t[:, tp * 128:(tp + 1) * 128], IDB[:])
                nc.vector.tensor_copy(
                    XC3[:, blk * 128:(blk + 1) * 128], ps[:]
                )

        product_stream(lambda s, b, sb: dr_lhs(XC2, sb),
                       b1_sink, pack_batches=True)

        if PHASES < 5:
            return nc
        # ---- B2: psum = 2^18*(S_s @ xc1)^T packed; t = psum*2^-17 - rs^T ----
        def b2_sink(s, b, j, c0, cnt, psum):
            cc = c0 + j
            cols = slice(cc * CH, (cc + 1) * CH)
            t = stage.tile([128, CH], BF16, tag="sg", name="sg")
            nc.vector.scalar_tensor_tensor(
                t[:], psum[:], 2.0 / (S_SCALE * X1_SCALE), RST[:, cols],
                op0=ALU.mult, op1=ALU.subtract,
            )
            nc.sync.dma_start(h2[s][1][:, cols], t[:])

        product_stream(lambda s, b, sb: dr_lhs(XC3, sb),
                       b2_sink, pack_batches=True)

        if PHASES < 6:
            return nc
        # ---- D2: dense c + tanh + blend + transpose + out ----
        for b in range(B2):
            for cc in range(NCH):
                cols = slice(cc * CH, (cc + 1) * CH)
                ps = pt.tile([128, CH], F32, tag="tp", name="tp")
                pc = ps[0:64, :]
                nmm = 0
                for m in range(5):
                    # inputs-half: lhsT at rows 0:64, rhs at base 0
                    if m == 0:
                        rhs_i = XT[b][0:64, cols]
                    else:
                        sg = stage.tile([128, CH], BF16, tag="sg", name="sg")
                        nc.sync.dma_start(sg[0:64, :], h1[b][m - 1][0:64, cols])
                        rhs_i = sg[0:64, :]
                    nc.tensor.matmul(
                        pc, WC[0:64, m * 64:(m + 1) * 64], rhs_i,
                        start=(nmm == 0), stop=False,
                    )
                    nmm += 1
                    # states-half: stage everything at base 0 so every matmul
                    # keeps tile_position (0,0)
                    if m == 0:
                        sgr = stage.tile([64, CH], BF16, tag="sgr", name="sgr",
                                         bufs=3)
                        nc.vector.tensor_copy(
                            sgr[:], RST[b * 64:(b + 1) * 64, cols]
                        )
                        rhs_s = sgr[:]
                    else:
                        s_idx = (m - 1) // 2
                        k_idx = (m - 1) % 2
                        sg = stage.tile([128, CH], BF16, tag="sg", name="sg")
                        nc.sync.dma_start(
                            sg[0:64, :],
                            h2[s_idx][k_idx][b * 64:(b + 1) * 64, cols],
                        )
                        rhs_s = sg[0:64, :]
                    lhs_s = WC[0:64, 320 + m * 64:320 + (m + 1) * 64]
                    nmm += 1
                    nc.tensor.matmul(
                        pc, lhs_s, rhs_s, start=False, stop=(nmm == 10),
                    )
                ctf = stage.tile([64, CH], F32, tag="f1", name="f1", bufs=3)
                nc.scalar.activation(ctf[:], pc, AF.Tanh, bias=BC[:])
                sts = stage.tile([64, CH], F32, tag="f5", name="f5", bufs=3)
                nc.vector.tensor_copy(sts[:], XT[b][64:128, cols])
                uts = stage.tile([64, CH], F32, tag="f6", name="f6", bufs=3)
                nc.vector.tensor_copy(uts[:], RUT[b][64:128, cols])
                t1 = stage.tile([64, CH], F32, tag="f2", name="f2", bufs=3)
                nc.vector.tensor_sub(t1[:], sts[:], ctf[:])
                t2 = stage.tile([64, CH], F32, tag="f3", name="f3", bufs=3)
                nc.vector.tensor_mul(t2[:], t1[:], uts[:])
                otf = stage.tile([64, CH], F32, tag="f4", name="f4", bufs=3)
                nc.vector.tensor_add(otf[:], ctf[:], t2[:])
                for tp in range(4):
                    blk = cc * 4 + tp
                    pso = pt.tile([128, 128], F32, tag="tp", name="tp")
                    nc.tensor.transpose(
                        pso[0:128, 0:64],
                        otf[:, tp * 128:(tp + 1) * 128],
                        IDF[0:64, 0:64],
                    )
                    ont = onat.tile([128, 64], F32, tag="on", name="on")
                    nc.vector.tensor_copy(ont[:], pso[0:128, 0:64])
                    nc.sync.dma_start(
                        out_ap[b, blk * 128:(blk + 1) * 128, :], ont[:]
                    )

    return nc


def _get_nc():
    if "nc" not in _CACHE:
        nc = _build()
        nc.compile()
        _CACHE["nc"] = nc
    return _CACHE["nc"]


def _prep_inputs(inputs, states, supports, W_ru, b_ru, W_c, b_c):
    bf = ml_dtypes.bfloat16
    e4 = ml_dtypes.float8_e4m3

    x_cat = np.concatenate([inputs, states], axis=-1).astype(np.float32)
    x_catT = np.ascontiguousarray(x_cat.transpose(0, 2, 1)).astype(bf)
    x_cat8 = x_cat.astype(e4)
    # supports^T *4096 -> e4m3, DoubleRow layout [s, sb, ki, ko, n]
    supT = np.asarray(supports).transpose(0, 2, 1) * np.float32(S_SCALE)
    supT = np.ascontiguousarray(
        supT.reshape(NSUP, NSB, 2, 128, N).transpose(0, 1, 3, 2, 4)
    ).astype(e4)
    # pre-compensate W rows of the *64-scaled hop-1 feature groups
    wru = np.asarray(W_ru).astype(np.float32).copy()
    wc = np.asarray(W_c).astype(np.float32).copy()
    for g in (1, 3):
        wru[g * DC:(g + 1) * DC] /= X1_SCALE
        wc[g * DC:(g + 1) * DC] /= X1_SCALE
    wru = wru.astype(bf)
    wc = wc.astype(bf)
    bru = np.asarray(b_ru).astype(np.float32).reshape(2 * H, 1)
    bc = np.asarray(b_c).astype(np.float32).reshape(H, 1)
    return x_catT, x_cat8, supT, wru, wc, bru, bc


def kernel(inputs, states, supports, W_ru, b_ru, W_c, b_c, _trace=False):
    B = inputs.shape[0]
    ncore = 8
    bper = B // ncore

    x_catT, x_cat8, supT, wru, wc, bru, bc = _prep_inputs(
        inputs, states, supports, W_ru, b_ru, W_c, b_c)

    nc = _get_nc()
    in_maps = []
    for c in range(ncore):
        in_maps.append({
            "xcatT": np.ascontiguousarray(x_catT[c * bper:(c + 1) * bper]),
            "xcat8": np.ascontiguousarray(x_cat8[c * bper:(c + 1) * bper]),
            "supT": supT,
            "wru": wru,
            "wc": wc,
            "bru": bru,
            "bc": bc,
        })
    res = run_bass_kernel_spmd(
        nc, in_maps, core_ids=list(range(ncore)), trace=_trace,
    )
    outs = [r["out"] for r in res.results]
    full = np.concatenate(outs, axis=0).astype(np.float32)           # [16,N,64]
    if _trace:
        kernel.last_results = res
    return full, full


# revision 52
# speedup vs baseline: 1.0270x; 1.0270x over previous
"""DCGRU cell Trainium2 kernel: batch-parallel SPMD over 8 NeuronCores.

Sharding: data-parallel over batch B=16 -> 2 batches/core; supports and
weights replicated. No collectives.

fp8 version: all S-product matmuls run fp8-e4m3 with DoubleRow perf mode
(2 contraction rows per PE cell -> 2x MACs/cycle, and half the support
DMA bytes vs bf16). Scales keep operands in e4m3 range:
  S stored as S*4096 (values ~U[0,1])
  x1 = S@x stored as 64*x1 (std ~0.8); W rows for x1 feature groups
  pre-divided by 64 on host. Dense (W) matmuls + activations stay bf16/f32.

Device algorithm per core (batches b0,b1):
  x = concat([inputs, states], -1)                  [N, 128] per batch
  Phase 0:  x^T (bf16) and x natural (fp8) DMA'd straight from host
  Phase A1: psum = 4096*(S_s @ x)^T; t = psum*2^-6 = 64*x1 (bf16) -> h1 DRAM;
            PE-transpose t -> X1 fp8 natural (value 64*x1)
  Phase A2: psum = 2^18*(S_s @ x1)^T; t = psum*2^-17 - x^T = x2^T -> h1
  Phase D1: ru^T = sigmoid(W_ru^T h^T + b), rs^T = r^T * states^T,
            XC2 fp8 natural (batch-packed 2x64 features)
  Phase B1: psum = 4096*(S_s @ rs)^T packed; t = psum*2^-6 -> h2 + XC3 fp8
  Phase B2: psum = 2^18*(S_s @ xc1)^T; t = psum*2^-17 - rs^T -> h2
  Phase D2: c^T = tanh(W_c^T h'^T + b_c), out^T = c + u*(s - c),
            PE-transpose -> natural, DMA out.

NOTE on pools: staging stays in one continuously-live `stage` pool with
per-tag uniform sizes, and all DMAs go on the sync queue. Variations
(per-phase pools, gpsimd-queue DMAs, cross-phase prefetch) trip
under-synchronized SBUF zone reuse (CoreSim race detector).
"""

import sys

sys.path.insert(0, "/opt/trn_rl_repo")

from contextlib import ExitStack

import ml_dtypes
import numpy as np

import concourse.bacc as bacc
import concourse.bass as bass
import concourse.mybir as mybir
import concourse.tile as tile
from concourse.bass_utils import run_bass_kernel_spmd

BF16 = mybir.dt.bfloat16
FP8 = mybir.dt.float8e4
F32 = mybir.dt.float32
AF = mybir.ActivationFunctionType
ALU = mybir.AluOpType
DR = mybir.MatmulPerfMode.DoubleRow

N = 8192
DC = 128          # D_IN + D_H
H = 64
B2 = 2            # batches per core
NSB = N // 256    # 32 contraction superblocks (DoubleRow: 256 rows each)
CH = 512          # psum chunk (free dim)
NCH = N // CH     # 16 chunks
# groups of chunks sharing one stationary load; 6 product psum banks max
GROUPS = [(i * 2, 2) for i in range(8)]
NSUP = 2

S_SCALE = 4096.0   # S stored *4096 in e4m3
X1_SCALE = 64.0    # hop-1 features stored *64

_CACHE = {}


def _build():
    import os
    PHASES = int(os.environ.get("DCGRU_PHASES", "6"))
    nc = bacc.Bacc("TRN2", target_bir_lowering=False, debug=False)

    xct_d = nc.dram_tensor("xcatT", [B2, DC, N], BF16, kind="ExternalInput")
    xc8_d = nc.dram_tensor("xcat8", [B2, N, DC], FP8, kind="ExternalInput")
    # supports^T, *4096, fp8, DoubleRow layout [s, sb, ki, ko, n]
    sup_d = nc.dram_tensor("supT", [NSUP, NSB, 128, 2, N], FP8,
                           kind="ExternalInput")
    wru_d = nc.dram_tensor("wru", [5 * DC, 2 * H], BF16, kind="ExternalInput")
    wc_d = nc.dram_tensor("wc", [5 * DC, H], BF16, kind="ExternalInput")
    bru_d = nc.dram_tensor("bru", [2 * H, 1], F32, kind="ExternalInput")
    bc_d = nc.dram_tensor("bc", [H, 1], F32, kind="ExternalInput")
    out_d = nc.dram_tensor("out", [B2, N, H], F32, kind="ExternalOutput")

    id_bf = nc.inline_tensor(np.eye(128, dtype=ml_dtypes.bfloat16), "id_bf")
    id_f = nc.inline_tensor(np.eye(128, dtype=np.float32), "id_f")

    xct_ap = xct_d.ap()
    xc8_ap = xc8_d.ap()
    sup_ap = sup_d.ap()
    out_ap = out_d.ap()

    with tile.TileContext(nc) as tc, ExitStack() as ctx:
        cpool = ctx.enter_context(tc.tile_pool(name="const", bufs=1))
        dram = ctx.enter_context(tc.tile_pool(name="dram", bufs=1, space="DRAM"))
        pers = ctx.enter_context(tc.tile_pool(name="pers", bufs=1))
        st = ctx.enter_context(tc.tile_pool(name="st", bufs=6))
        stage = ctx.enter_context(tc.tile_pool(name="stage", bufs=10))
        onat = ctx.enter_context(tc.tile_pool(name="onat", bufs=4))
        pp = ctx.enter_context(tc.tile_pool(name="pp", bufs=6, space="PSUM"))
        pt = ctx.enter_context(tc.tile_pool(name="pt", bufs=2, space="PSUM"))

        # ---- constants ----
        IDB = cpool.tile([128, 128], BF16, tag="idb", name="idb")
        nc.sync.dma_start(IDB[:], id_bf.ap())
        IDF = cpool.tile([128, 128], F32, tag="idf", name="idf")
        nc.sync.dma_start(IDF[:], id_f.ap())
        WRU = cpool.tile([128, 5 * 128], BF16, tag="wru", name="wru")
        nc.sync.dma_start(
            WRU[:].rearrange("p (a o) -> p a o", a=5),
            wru_d.ap().rearrange("(a p) o -> p a o", p=128),
        )
        # WC layout: cols m*64:(m+1)*64 = inputs-half block (rows 0:64);
        # cols 320+m*64 = states-half block, duplicated at rows 0:64 and 64:128
        WC = cpool.tile([128, 10 * 64], BF16, tag="wc", name="wc")
        for m in range(5):
            nc.sync.dma_start(
                WC[0:64, m * 64:(m + 1) * 64], wc_d.ap()[m * 128:m * 128 + 64, :]
            )
            nc.sync.dma_start(
                WC[0:64, 320 + m * 64:320 + (m + 1) * 64],
                wc_d.ap()[m * 128 + 64:(m + 1) * 128, :],
            )
            nc.sync.dma_start(
                WC[64:128, 320 + m * 64:320 + (m + 1) * 64],
                wc_d.ap()[m * 128 + 64:(m + 1) * 128, :],
            )
        BRU = cpool.tile([128, 1], F32, tag="bru", name="bru")
        nc.sync.dma_start(BRU[:], bru_d.ap())
        BC = cpool.tile([64, 1], F32, tag="bc", name="bc")
        nc.sync.dma_start(BC[:], bc_d.ap())

        # ---- DRAM scratch: gconv1 product feats^T (x1_s0, x2_s0, x1_s1, x2_s1) ----
        h1 = [[dram.tile([128, N], BF16, tag=f"h1_{b}_{m}", name=f"h1_{b}_{m}") for m in range(4)]
              for b in range(B2)]
        # gconv2 states-half feats^T, batch-packed rows (b*64): [x1'_s, x2'_s]
        h2 = [[dram.tile([128, N], BF16, tag=f"h2_{s}_{k}", name=f"h2_{s}_{k}") for k in range(2)]
              for s in range(NSUP)]

        # ---- persistent SBUF; tags share slots across phases by lifetime ----
        # big4 (fp8): X1 (A1-A2) -> XC2/XC3 (D1-B2)
        XT = [pers.tile([128, N], BF16, tag="xt", name=f"XT_{b}", bufs=2)
              for b in range(B2)]
        X8 = [pers.tile([128, N], FP8, tag="x8", name=f"X8_{b}", bufs=2)
              for b in range(B2)]
        X1 = [[pers.tile([128, N], FP8, tag="big4", name=f"X1_{s}_{b}", bufs=4)
               for b in range(B2)] for s in range(NSUP)]

        def dr_lhs(tile_, sb):
            """[128, 2, 128] DoubleRow stationary view of natural fp8 tile."""
            return tile_[:, sb * 256:(sb + 1) * 256].rearrange(
                "p (i d) -> p i d", i=2)

        # ---- phase 0: x^T (bf16) and x natural (fp8) come straight from host ----
        for b in range(B2):
            nc.sync.dma_start(XT[b][:], xct_ap[b])
            nc.sync.dma_start(
                X8[b][:].rearrange("p (a d) -> p a d", a=N // 128),
                xc8_ap[b].rearrange("(a p) d -> p a d", p=128),
            )

        def product_stream(lhs_of, psum_sink, pack_batches):
            """Stream supT once; for each (s, group, superblock) do DR matmuls.

            lhs_of(s, b, sb) -> lhsT AP [128, 2, 128] fp8. psum_sink(s,
            b_or_None, j, c0, cnt, psum) consumes the finished [128, CH]
            f32 psum for chunk c0+j.
            """
            for s in range(NSUP):
                for (c0, cnt) in GROUPS:
                    gc = cnt * CH
                    if pack_batches:
                        psums = [pp.tile([128, CH], F32, tag="pp", name="pp") for j in range(cnt)]
                    else:
                        psums = [pp.tile([128, CH], F32, tag="pp", name="pp")
                                 for _ in range(B2 * cnt)]
                    for sb in range(NSB):
                        stt = st.tile([128, 2, gc], FP8, tag="st", name="st")
                        nc.sync.dma_start(
                            stt[:],
                            sup_ap[s, sb, :, :, c0 * CH:c0 * CH + gc],
                        )
                        first = sb == 0
                        last = sb == NSB - 1
                        if pack_batches:
                            lhsT = lhs_of(s, None, sb)
                            for j in range(cnt):
                                nc.tensor.matmul(
                                    psums[j][:], lhsT,
                                    stt[:, :, j * CH:(j + 1) * CH],
                                    start=first, stop=last, perf_mode=DR,
                                )
                        else:
                            for b in range(B2):
                                lhsT = lhs_of(s, b, sb)
                                for j in range(cnt):
                                    nc.tensor.matmul(
                                        psums[b * cnt + j][:], lhsT,
                                        stt[:, :, j * CH:(j + 1) * CH],
                                        start=first, stop=last, perf_mode=DR,
                                    )
                    if pack_batches:
                        for j in range(cnt):
                            psum_sink(s, None, j, c0, cnt, psums[j])
                    else:
                        for b in range(B2):
                            for j in range(cnt):
                                psum_sink(s, b, j, c0, cnt, psums[b * cnt + j])

        # ---- A1: psum = 4096*(S_s @ x)^T; t = 64*x1 ----
        def a1_sink(s, b, j, c0, cnt, psum):
            cc = c0 + j
            cols = slice(cc * CH, (cc + 1) * CH)
            t = stage.tile([128, CH], BF16, tag="sg", name="sg")
            nc.scalar.activation(t[:], psum[:], AF.Copy, scale=1.0 / X1_SCALE)
            nc.sync.dma_start(h1[b][2 * s][:, cols], t[:])
            for tp in range(4):
                blk = cc * 4 + tp
                ps = pt.tile([128, 128], BF16, tag="tp", name="tp")
                nc.tensor.transpose(ps[:], t[:, tp * 128:(tp + 1) * 128], IDB[:])
                nc.vector.tensor_copy(
                    X1[s][b][:, blk * 128:(blk + 1) * 128], ps[:]
                )

        product_stream(lambda s, b, sb: dr_lhs(X8[b], sb),
                       a1_sink, pack_batches=False)

        if PHASES < 2:
            return nc
        # ---- A2: psum = 2^18*(S_s @ x1)^T; t = psum*2^-17 - x^T ----
        def a2_sink(s, b, j, c0, cnt, psum):
            cc = c0 + j
            cols = slice(cc * CH, (cc + 1) * CH)
            t = stage.tile([128, CH], BF16, tag="sg", name="sg")
            nc.vector.scalar_tensor_tensor(
                t[:], psum[:], 2.0 / (S_SCALE * X1_SCALE), XT[b][:, cols],
                op0=ALU.mult, op1=ALU.subtract,
            )
            nc.sync.dma_start(h1[b][2 * s + 1][:, cols], t[:])

        product_stream(lambda s, b, sb: dr_lhs(X1[s][b], sb),
                       a2_sink, pack_batches=False)

        if PHASES < 3:
            return nc
        # ---- D1: dense ru + sigmoid + rs^T + XC2 fp8 natural ----
        RUT = [pers.tile([128, N], BF16, tag="big2", name=f"RUT_{b}", bufs=2)
               for b in range(B2)]
        RST = pers.tile([128, N], BF16, tag="rst", name="RST", bufs=1)
        XC2 = pers.tile([128, N], FP8, tag="big4", name="XC2", bufs=4)
        for cc in range(NCH):
            for b in range(B2):
                cols = slice(cc * CH, (cc + 1) * CH)
                ps = pt.tile([128, CH], F32, tag="tp", name="tp")
                for i in range(5):
                    if i == 0:
                        rhs = XT[b][:, cols]
                    else:
                        sg = stage.tile([128, CH], BF16, tag="sg", name="sg")
                        nc.sync.dma_start(sg[:], h1[b][i - 1][:, cols])
                        rhs = sg[:]
                    nc.tensor.matmul(
                        ps[:], WRU[:, i * 128:(i + 1) * 128], rhs,
                        start=(i == 0), stop=(i == 4),
                    )
                nc.scalar.activation(
                    RUT[b][:, cols], ps[:], AF.Sigmoid, bias=BRU[:]
                )
                # rs = r * states^T; base-shift states^T and the result via
                # single-input copies (SB-SB two-input ops need equal bases)
                sts = stage.tile([64, CH], BF16, tag="sh1", name="sh1", bufs=3)
                nc.vector.tensor_copy(sts[:], XT[b][64:128, cols])
                rsc = stage.tile([64, CH], BF16, tag="sh2", name="sh2", bufs=3)
                nc.vector.tensor_mul(rsc[:], RUT[b][0:64, cols], sts[:])
                nc.vector.tensor_copy(RST[b * 64:(b + 1) * 64, cols], rsc[:])
                if b == 1:
                    # both batches' rs written for this chunk: XC2 natural via
                    # one packed transpose per 128-node block
                    for tp in range(4):
                        blk = cc * 4 + tp
                        ps2 = pt.tile([128, 128], BF16, tag="tp", name="tp")
                        nc.tensor.transpose(
                            ps2[:], RST[:, blk * 128:(blk + 1) * 128], IDB[:]
                        )
                        nc.vector.tensor_copy(
                            XC2[:, blk * 128:(blk + 1) * 128], ps2[:]
                        )

        if PHASES < 4:
            return nc
        # ---- B1: psum = 4096*(S_s @ rs)^T packed; t = 64*xc1 ----
        XC3 = pers.tile([128, N], FP8, tag="big4", name="XC3", bufs=4)

        def b1_sink(s, b, j, c0, cnt, psum):
            cc = c0 + j
            cols = slice(cc * CH, (cc + 1) * CH)
            t = stage.tile([128, CH], BF16, tag="sg", name="sg")
            nc.scalar.activation(t[:], psum[:], AF.Copy, scale=1.0 / X1_SCALE)
            nc.sync.dma_start(h2[s][0][:, cols], t[:])
            for tp in range(4):
                blk = cc * 4 + tp
                ps = pt.tile([128, 128], BF16, tag="tp", name="tp")
                nc.tensor.transpose(ps[:], t[:, tp * 128:(tp + 1) * 128], IDB[:])
                nc.vector.tensor_copy(
                    XC3[:, blk * 128:(blk + 1) * 128], ps[:]
                )

        product_stream(lambda s, b, sb: dr_lhs(XC2, sb),
                       b1_sink, pack_batches=True)

        if PHASES < 5:
            return nc
        # ---- B2: psum = 2^18*(S_s @ xc1)^T packed; t = psum*2^-17 - rs^T ----
        def b2_sink(s, b, j, c0, cnt, psum):
            cc = c0 + j
            cols = slice(cc * CH, (cc + 1) * CH)
            t = stage.tile([128, CH], BF16, tag="sg", name="sg")
            nc.vector.scalar_tensor_tensor(
                t[:], psum[:], 2.0 / (S_SCALE * X1_SCALE), RST[:, cols],
                op0=ALU.mult, op1=ALU.subtract,
            )
            nc.sync.dma_start(h2[s][1][:, cols], t[:])

        product_stream(lambda s, b, sb: dr_lhs(XC3, sb),
                       b2_sink, pack_batches=True)

        if PHASES < 6:
            return nc
        # ---- D2: dense c + tanh + blend + transpose + out ----
        for cc in range(NCH):
            for b in range(B2):
                cols = slice(cc * CH, (cc + 1) * CH)
                ps = pt.tile([128, CH], F32, tag="tp", name="tp")
                pc = ps[0:64, :]
                nmm = 0
                for m in range(5):
                    # inputs-half: lhsT at rows 0:64, rhs at base 0
                    if m == 0:
                        rhs_i = XT[b][0:64, cols]
                    else:
                        sg = stage.tile([128, CH], BF16, tag="sg", name="sg")
                        nc.sync.dma_start(sg[0:64, :], h1[b][m - 1][0:64, cols])
                        rhs_i = sg[0:64, :]
                    nc.tensor.matmul(
                        pc, WC[0:64, m * 64:(m + 1) * 64], rhs_i,
                        start=(nmm == 0), stop=False,
                    )
                    nmm += 1
                    # states-half: stage everything at base 0 so every matmul
                    # keeps tile_position (0,0)
                    if m == 0:
                        sgr = stage.tile([64, CH], BF16, tag="sgr", name="sgr",
                                         bufs=3)
                        nc.vector.tensor_copy(
                            sgr[:], RST[b * 64:(b + 1) * 64, cols]
                        )
                        rhs_s = sgr[:]
                    else:
                        s_idx = (m - 1) // 2
                        k_idx = (m - 1) % 2
                        sg = stage.tile([128, CH], BF16, tag="sg", name="sg")
                        nc.sync.dma_start(
                            sg[0:64, :],
                            h2[s_idx][k_idx][b * 64:(b + 1) * 64, cols],
                        )
                        rhs_s = sg[0:64, :]
                    lhs_s = WC[0:64, 320 + m * 64:320 + (m + 1) * 64]
                    nmm += 1
                    nc.tensor.matmul(
                        pc, lhs_s, rhs_s, start=False, stop=(nmm == 10),
                    )
                ctf = stage.tile([64, CH], F32, tag="f1", name="f1", bufs=3)
                nc.scalar.activation(ctf[:], pc, AF.Tanh, bias=BC[:])
                sts = stage.tile([64, CH], F32, tag="f5", name="f5", bufs=3)
                nc.vector.tensor_copy(sts[:], XT[b][64:128, cols])
                uts = stage.tile([64, CH], F32, tag="f6", name="f6", bufs=3)
                nc.vector.tensor_copy(uts[:], RUT[b][64:128, cols])
                t1 = stage.tile([64, CH], F32, tag="f2", name="f2", bufs=3)
                nc.vector.tensor_sub(t1[:], sts[:], ctf[:])
                t2 = stage.tile([64, CH], F32, tag="f3", name="f3", bufs=3)
                nc.vector.tensor_mul(t2[:], t1[:], uts[:])
                otf = stage.tile([64, CH], F32, tag="f4", name="f4", bufs=3)
                nc.vector.tensor_add(otf[:], ctf[:], t2[:])
                for tp in range(4):
                    blk = cc * 4 + tp
                    pso = pt.tile([128, 128], F32, tag="tp", name="tp")
                    nc.tensor.transpose(
                        pso[0:128, 0:64],
                        otf[:, tp * 128:(tp + 1) * 128],
                        IDF[0:64, 0:64],
                    )
                    ont = onat.tile([128, 64], F32, tag="on", name="on")
                    nc.vector.tensor_copy(ont[:], pso[0:128, 0:64])
                    nc.sync.dma_start(
                        out_ap[b, blk * 128:(blk + 1) * 128, :], ont[:]
                    )

    return nc


def _get_nc():
    if "nc" not in _CACHE:
        nc = _build()
        nc.compile()
        _CACHE["nc"] = nc
    return _CACHE["nc"]


def _prep_inputs(inputs, states, supports, W_ru, b_ru, W_c, b_c):
    bf = ml_dtypes.bfloat16
    e4 = ml_dtypes.float8_e4m3

    x_cat = np.concatenate([inputs, states], axis=-1).astype(np.float32)
    x_catT = np.ascontiguousarray(x_cat.transpose(0, 2, 1)).astype(bf)
    x_cat8 = x_cat.astype(e4)
    # supports^T *4096 -> e4m3, DoubleRow layout [s, sb, ki, ko, n]
    supT = np.asarray(supports).transpose(0, 2, 1) * np.float32(S_SCALE)
    supT = np.ascontiguousarray(
        supT.reshape(NSUP, NSB, 2, 128, N).transpose(0, 1, 3, 2, 4)
    ).astype(e4)
    # pre-compensate W rows of the *64-scaled hop-1 feature groups
    wru = np.asarray(W_ru).astype(np.float32).copy()
    wc = np.asarray(W_c).astype(np.float32).copy()
    for g in (1, 3):
        wru[g * DC:(g + 1) * DC] /= X1_SCALE
        wc[g * DC:(g + 1) * DC] /= X1_SCALE
    wru = wru.astype(bf)
    wc = wc.astype(bf)
    bru = np.asarray(b_ru).astype(np.float32).reshape(2 * H, 1)
    bc = np.asarray(b_c).astype(np.float32).reshape(H, 1)
    return x_catT, x_cat8, supT, wru, wc, bru, bc


def kernel(inputs, states, supports, W_ru, b_ru, W_c, b_c, _trace=False):
    B = inputs.shape[0]
    ncore = 8
    bper = B // ncore

    x_catT, x_cat8, supT, wru, wc, bru, bc = _prep_inputs(
        inputs, states, supports, W_ru, b_ru, W_c, b_c)

    nc = _get_nc()
    in_maps = []
    for c in range(ncore):
        in_maps.append({
            "xcatT": np.ascontiguousarray(x_catT[c * bper:(c + 1) * bper]),
            "xcat8": np.ascontiguousarray(x_cat8[c * bper:(c + 1) * bper]),
            "supT": supT,
            "wru": wru,
            "wc": wc,
            "bru": bru,
            "bc": bc,
        })
    res = run_bass_kernel_spmd(
        nc, in_maps, core_ids=list(range(ncore)), trace=_trace,
    )
    outs = [r["out"] for r in res.results]
    full = np.concatenate(outs, axis=0).astype(np.float32)           # [16,N,64]
    if _trace:
        kernel.last_results = res
    return full, full
